# revision 2
# baseline (speedup 1.0000x reference)
"""Binarized 3x3 conv + bias + ReLU + eval-mode BatchNorm, Trainium2 Bass kernel.

Problem: x[16,64,256,256] f32, w[64,64,3,3], per-channel b/gamma/beta/mean/var.
  y = BN(relu(conv(sign(x), sign(w)) + b))  (eval-mode BN = per-channel affine)

Strategy (8 NeuronCores, data-parallel over batch):
  - 2 images per core; image A on SBUF partitions 0-63 (channels), image B on 64-127.
  - Binarize on-chip as t = (x >= 0) in {1,0} bf16 (one DVE is_ge op); spatial
    padding uses 0.5 so that the identity  conv_pm = 2*conv_t - S  holds exactly
    (S[co] = sum of sign(w) over taps; pads contribute 2*0.5-1 = 0).
  - 3x3 conv = 9 accumulating matmuls per PSUM tile (K=Cin=64, M=Cout=64),
    using 64x64 PE array tiling: 4 quadrants = (imgA,imgB) x (left,right 128-col
    half) run concurrently -> full 128x128 array utilization.
  - Post: ScalarE relu(2*psum + (b-S)) then VectorE y = t*inv + c, both with
    per-partition vectors.
  - ALL DMAs are 128-partition HWDGE: input rows on the SP ring (nc.sync),
    output rows on the ACT ring (nc.scalar).  No SWDGE/gpsimd DMAs (they
    starve on the shared SBUF port while DVE runs and halve throughput).
  - PSUM bank B holds image-swapped halves (PE quadrant packing); its rows are
    written to the *other* image's row range in DRAM and the swap is undone on
    the host during the gather (free - not on the device critical path).
  - Input halo rows are NOT re-read from HBM: the 2 boundary rows of each
    block are copied (GpSimd) from the previous block's binarized tile.
"""

import numpy as np
import ml_dtypes

import concourse.bass as bass  # noqa: F401  (AP types ride along)
import concourse.mybir as mybir
import concourse.tile as tile
from concourse import bacc
from concourse.bass_utils import run_bass_kernel_spmd

N_CORES = 8
IMGS_PER_CORE = 2
C = 64
H = 256
W = 256
RB = 32              # output rows per block
NBLK = H // RB       # 8
ROWS_IN = RB + 2     # buffer rows: halo row above + 32 outputs + halo row below
WP = W + 4           # padded row width in xb; data at col offset 2
BN_EPS = 1e-5
DT = mybir.dt

_PROGRAM = None


def _build():
    nc = bacc.Bacc(
        "TRN2",
        target_bir_lowering=False,
        debug=False,
        enable_asserts=False,
    )
    x = nc.dram_tensor("x", [IMGS_PER_CORE, C, H, W], DT.float32, kind="ExternalInput")
    wT = nc.dram_tensor("wT", [128, 9 * 64], DT.bfloat16, kind="ExternalInput")
    bvec = nc.dram_tensor("bvec", [128, 1], DT.float32, kind="ExternalInput")
    ivec = nc.dram_tensor("ivec", [128, 1], DT.float32, kind="ExternalInput")
    cvec = nc.dram_tensor("cvec", [128, 1], DT.float32, kind="ExternalInput")
    y = nc.dram_tensor("y", [IMGS_PER_CORE, C, H, W], DT.float32, kind="ExternalOutput")

    x_flat = x.ap().rearrange("n c h w -> (n c) (h w)")   # [128, 65536] flat
    y_m = y.ap().rearrange("n c h w -> (n c) (h w)")      # [128, 65536] flat

    with tile.TileContext(nc) as tc:
        with (
            tc.tile_pool(name="consts", bufs=1) as cpool,
            tc.tile_pool(name="xin", bufs=2) as xpool,
            tc.tile_pool(name="xbp", bufs=2) as xbpool,
            tc.tile_pool(name="tsb", bufs=4) as tpool,
            tc.tile_pool(name="yout", bufs=2) as ypool,
            tc.tile_pool(name="psum", bufs=2, space="PSUM") as ppool,
        ):
            wt = cpool.tile([128, 9 * 64], DT.bfloat16, tag="wt")
            bv = cpool.tile([128, 1], DT.float32, tag="bv")
            iv = cpool.tile([128, 1], DT.float32, tag="iv")
            cv = cpool.tile([128, 1], DT.float32, tag="cv")

            def load_consts():
                nc.sync.dma_start(wt[:], wT.ap())
                nc.sync.dma_start(bv[:], bvec.ap())
                nc.sync.dma_start(iv[:], ivec.ap())
                nc.sync.dma_start(cv[:], cvec.ap())

            def load_block(blk, xb_prev):
                """DMA this block's NEW input rows, binarize into the padded
                bf16 tile; halo rows come from the previous block's tile.

                xb row k  <->  x row  blk*RB - 1 + k   (k = 0..33), so matmul
                for output row j, tap dy reads xb rows j+dy, j+dy+1.
                """
                r0 = blk * RB
                xin = xpool.tile([128, 33 * W], DT.float32, tag="xin")
                xin_v = xin[:].rearrange("p (r c) -> p r c", c=W)
                xb = xbpool.tile([128, ROWS_IN * WP], DT.bfloat16, tag="xb")
                xb_v = xb[:].rearrange("p (r c) -> p r c", c=WP)

                if blk == 0:
                    # xb rows 1..33  <-  x rows 0..32 (33 rows), split in two
                    # chunks so binarize+matmul can start before the second
                    # chunk lands.  xb row 0 = top pad.
                    for a, b in ((0, 17), (17, 33)):
                        nc.sync.dma_start(
                            xin[:, a * W : b * W], x_flat[:, a * W : b * W]
                        )
                        nc.vector.tensor_scalar(
                            xb_v[:, 1 + a : 1 + b, 2 : 2 + W],
                            xin_v[:, a:b, :],
                            0.0,
                            None,
                            op0=mybir.AluOpType.is_ge,
                        )
                    nc.gpsimd.memset(xb_v[:, 0:1, :], 0.5)
                    nc.gpsimd.memset(xb_v[:, 1:34, 0:2], 0.5)
                    nc.gpsimd.memset(xb_v[:, 1:34, 2 + W : WP], 0.5)
                else:
                    # halo: xb rows 0,1  <-  previous tile rows 32,33 (with
                    # their column pads already in place)
                    nc.gpsimd.tensor_copy(xb_v[:, 0:2, :], xb_prev[:, 32:34, :])
                    # new rows r0+1 .. r0+32 (31 rows + pad for last block)
                    n_new = 31 if blk == NBLK - 1 else 32
                    nc.sync.dma_start(
                        xin[:, 0 : n_new * W],
                        x_flat[:, (r0 + 1) * W : (r0 + 1 + n_new) * W],
                    )
                    nc.vector.tensor_scalar(
                        xb_v[:, 2 : 2 + n_new, 2 : 2 + W],
                        xin_v[:, 0:n_new, :],
                        0.0,
                        None,
                        op0=mybir.AluOpType.is_ge,
                    )
                    nc.gpsimd.memset(xb_v[:, 2:34, 0:2], 0.5)
                    nc.gpsimd.memset(xb_v[:, 2:34, 2 + W : WP], 0.5)
                    if blk == NBLK - 1:
                        nc.gpsimd.memset(xb_v[:, 33:34, :], 0.5)
                return xb_v

            def compute_block(blk, xb_v):
                """Matmuls + post-ops + output DMAs for a loaded block."""
                r0 = blk * RB
                hb = RB // 2  # rows per half-block (16)
                # PSUM bank T = [imgA-top | imgB-top] (partition = n*64+c);
                # bank B = [imgB-bot | imgA-bot] (image-reversed; the reversal
                # is undone host-side).
                # Quadrants: A-T=(0,0)  B-T=(64,64)  B-B=(64,0)  A-B=(0,64)
                yt_ = ypool.tile([128, hb * W], DT.float32, tag="ytop")
                yb_ = ypool.tile([128, hb * W], DT.float32, tag="ybot")
                split = 2 if blk == NBLK - 1 else 1
                for it2 in range(hb // 4):          # 4 output rows per super-tile
                    # super-tiles spanning 2 PSUM banks; each matmul stays in one
                    ps_t = ppool.tile([128, 1024], DT.float32, tag="pst")
                    ps_b = ppool.tile([128, 1024], DT.float32, tag="psb")
                    for sub in range(2):            # 2 rows per matmul set
                        it = 2 * it2 + sub
                        c0 = sub * 512
                        for t in range(9):
                            dy, dx = divmod(t, 3)
                            first, last = (t == 0), (t == 8)
                            rt = 2 * it + dy              # top-half rows
                            rb_ = hb + 2 * it + dy        # bottom-half rows
                            cs = 1 + dx
                            quads = (
                                (ps_t, 0, 0, rt),      # A-top -> psT[0:64]
                                (ps_t, 64, 64, rt),    # B-top -> psT[64:128]
                                (ps_b, 64, 0, rb_),    # B-bot -> psB[0:64]
                                (ps_b, 0, 64, rb_),    # A-bot -> psB[64:128]
                            )
                            for ps, xp0, op0_, rlo in quads:
                                wslc = wt[xp0 : xp0 + 64, t * 64 : (t + 1) * 64]
                                rhs = xb_v[xp0 : xp0 + 64, rlo : rlo + 2, cs : cs + W]
                                nc.tensor.matmul(
                                    ps[op0_ : op0_ + 64, c0 : c0 + 512],
                                    wslc,
                                    rhs,
                                    start=first,
                                    stop=last,
                                )
                    for ps, yst in ((ps_t, yt_), (ps_b, yb_)):
                        tsb = tpool.tile([128, 1024], DT.float32, tag="tsb")
                        nc.scalar.activation(
                            tsb[:],
                            ps[:],
                            mybir.ActivationFunctionType.Relu,
                            bias=bv[:],
                            scale=2.0,
                        )
                        nc.vector.tensor_scalar(
                            yst[:, it2 * 1024 : (it2 + 1) * 1024],
                            tsb[:],
                            iv[:],
                            cv[:],
                            op0=mybir.AluOpType.mult,
                            op1=mybir.AluOpType.add,
                        )
                    if split == 2 and it2 == 1:
                        # last block: flush the first half-tiles early so the
                        # tail drain overlaps the remaining compute
                        nc.scalar.dma_start(
                            y_m[:, r0 * W : (r0 + 8) * W], yt_[:, 0 : 8 * W]
                        )
                        nc.scalar.dma_start(
                            y_m[:, (r0 + hb) * W : (r0 + hb + 8) * W],
                            yb_[:, 0 : 8 * W],
                        )
                lo = 8 * W if split == 2 else 0
                nc.scalar.dma_start(
                    y_m[:, r0 * W + lo : (r0 + hb) * W], yt_[:, lo : hb * W]
                )
                nc.scalar.dma_start(
                    y_m[:, (r0 + hb) * W + lo : (r0 + RB) * W], yb_[:, lo : hb * W]
                )

            # software pipeline: queue block i+1's input DMA before block i's
            # output DMAs so input transfer overlaps compute
            pending = None
            prev_xb = None
            for blk in range(NBLK):
                xb_v = load_block(blk, prev_xb)
                if blk == 0:
                    load_consts()
                if pending is not None:
                    compute_block(pending[0], pending[1])
                pending = (blk, xb_v)
                prev_xb = xb_v
            compute_block(pending[0], pending[1])
    nc.compile()
    return nc


def _get_program():
    global _PROGRAM
    if _PROGRAM is None:
        _PROGRAM = _build()
    return _PROGRAM


def _prep_params(w, b, gamma, beta, running_mean, running_var):
    wb = np.where(w >= 0, 1.0, -1.0).astype(np.float32)          # [co, ci, ky, kx]
    wt = np.ascontiguousarray(wb.transpose(1, 2, 3, 0))          # [ci, ky, kx, co]
    wt = wt.reshape(C, 9 * C).astype(ml_dtypes.bfloat16)
    wt2 = np.ascontiguousarray(np.concatenate([wt, wt], axis=0))  # [128, 576]
    s = wb.sum(axis=(1, 2, 3)).astype(np.float32)
    inv = (gamma.astype(np.float32) / np.sqrt(running_var.astype(np.float32) + BN_EPS)).astype(np.float32)
    cc = (beta.astype(np.float32) - running_mean.astype(np.float32) * inv).astype(np.float32)
    bp = (b.astype(np.float32) - s).astype(np.float32)

    def rep(v):
        return np.ascontiguousarray(np.tile(v.astype(np.float32), 2).reshape(128, 1))

    return wt2, rep(bp), rep(inv), rep(cc)


def _unswizzle(yd):
    """Undo the on-device image swap of the bottom 16-row half of each
    32-row block (PSUM bank B holds image-reversed partitions)."""
    v = yd.reshape(IMGS_PER_CORE, C, NBLK, 2, RB // 2, W)
    out = np.empty_like(v)
    out[:, :, :, 0] = v[:, :, :, 0]
    out[:, :, :, 1] = v[::-1, :, :, 1]
    return out.reshape(IMGS_PER_CORE, C, H, W)


def run(x, w, b, gamma, beta, running_mean, running_var, trace=False):
    nc = _get_program()
    wt2, bp, inv, cc = _prep_params(w, b, gamma, beta, running_mean, running_var)
    x = np.asarray(x, dtype=np.float32)
    in_maps = []
    for i in range(N_CORES):
        in_maps.append(
            {
                "x": np.ascontiguousarray(x[IMGS_PER_CORE * i : IMGS_PER_CORE * (i + 1)]),
                "wT": wt2,
                "bvec": bp,
                "ivec": inv,
                "cvec": cc,
            }
        )
    res = run_bass_kernel_spmd(nc, in_maps, list(range(N_CORES)), trace=trace)
    y = np.concatenate(
        [_unswizzle(res.results[i]["y"]) for i in range(N_CORES)], axis=0
    )
    return y, res


def kernel(x, w, b, gamma, beta, running_mean, running_var):
    y, _ = run(x, w, b, gamma, beta, running_mean, running_var)
    return y


# revision 3
# speedup vs baseline: 1.1176x; 1.1176x over previous
"""Binarized 3x3 conv + bias + ReLU + eval-mode BatchNorm, Trainium2 Bass kernel.

Problem: x[16,64,256,256] f32, w[64,64,3,3], per-channel b/gamma/beta/mean/var.
  y = BN(relu(conv(sign(x), sign(w)) + b))  (eval-mode BN = per-channel affine)

Strategy (8 NeuronCores, data-parallel over batch):
  - 2 images per core; image A on SBUF partitions 0-63 (channels), image B on 64-127.
  - Binarize on-chip as t = (x >= 0) in {1,0} bf16 (one DVE is_ge op); spatial
    padding uses 0.5 so that the identity  conv_pm = 2*conv_t - S  holds exactly
    (S[co] = sum of sign(w) over taps; pads contribute 2*0.5-1 = 0).
  - 3x3 conv = 9 accumulating matmuls per PSUM tile (K=Cin=64, M=Cout=64),
    using 64x64 PE array tiling: 4 quadrants = (imgA,imgB) x (left,right 128-col
    half) run concurrently -> full 128x128 array utilization.
  - Post: ScalarE relu(2*psum + (b-S)) then VectorE y = t*inv + c, both with
    per-partition vectors, into per-supertile staging tiles (fine-grained
    back-pressure: DVE never blocks more than one 1 MB DMA deep).
  - ALL DMAs are 128-partition HWDGE.  Input rows ride the ACT ring
    (nc.scalar; their dispatch never waits, so ScalarE compute is not
    head-of-line blocked), output tiles ride the otherwise-idle SP ring
    (nc.sync; their ring-credit waits block nothing).  No SWDGE/gpsimd DMAs
    (they starve on the shared SBUF port while DVE runs).
  - PSUM bank B holds image-swapped halves (PE quadrant packing); its rows are
    written to the *other* image's row range in DRAM and the swap is undone on
    the host during the gather (free - not on the device critical path).
  - Input halo rows are NOT re-read from HBM: the 2 boundary rows of each
    block are copied (DVE, bf16) from the previous block's binarized tile.
"""

import numpy as np
import ml_dtypes

import concourse.bass as bass  # noqa: F401  (AP types ride along)
import concourse.mybir as mybir
import concourse.tile as tile
from concourse import bacc
from concourse.bass_utils import run_bass_kernel_spmd

N_CORES = 8
IMGS_PER_CORE = 2
C = 64
H = 256
W = 256
RB = 32              # output rows per block
NBLK = H // RB       # 8
ROWS_IN = RB + 2     # buffer rows: halo row above + 32 outputs + halo row below
WP = W + 4           # padded row width in xb; data at col offset 2
BN_EPS = 1e-5
DT = mybir.dt

_PROGRAM = None


def _build():
    nc = bacc.Bacc(
        "TRN2",
        target_bir_lowering=False,
        debug=False,
        enable_asserts=False,
    )
    x = nc.dram_tensor("x", [IMGS_PER_CORE, C, H, W], DT.float32, kind="ExternalInput")
    wT = nc.dram_tensor("wT", [128, 9 * 64], DT.bfloat16, kind="ExternalInput")
    bvec = nc.dram_tensor("bvec", [128, 1], DT.float32, kind="ExternalInput")
    ivec = nc.dram_tensor("ivec", [128, 1], DT.float32, kind="ExternalInput")
    cvec = nc.dram_tensor("cvec", [128, 1], DT.float32, kind="ExternalInput")
    y = nc.dram_tensor("y", [IMGS_PER_CORE, C, H, W], DT.float32, kind="ExternalOutput")

    x_flat = x.ap().rearrange("n c h w -> (n c) (h w)")   # [128, 65536] flat
    y_m = y.ap().rearrange("n c h w -> (n c) (h w)")      # [128, 65536] flat

    with tile.TileContext(nc) as tc:
        with (
            tc.tile_pool(name="consts", bufs=1) as cpool,
            tc.tile_pool(name="xin", bufs=2) as xpool,
            tc.tile_pool(name="xbp", bufs=2) as xbpool,
            tc.tile_pool(name="tsb", bufs=4) as tpool,
            tc.tile_pool(name="yout", bufs=10) as ypool,
            tc.tile_pool(name="psum", bufs=2, space="PSUM") as ppool,
        ):
            wt = cpool.tile([128, 9 * 64], DT.bfloat16, tag="wt")
            bv = cpool.tile([128, 1], DT.float32, tag="bv")
            iv = cpool.tile([128, 1], DT.float32, tag="iv")
            cv = cpool.tile([128, 1], DT.float32, tag="cv")

            def load_consts():
                nc.sync.dma_start(wt[:], wT.ap())
                nc.sync.dma_start(bv[:], bvec.ap())
                nc.sync.dma_start(iv[:], ivec.ap())
                nc.sync.dma_start(cv[:], cvec.ap())

            def load_block(blk, xb_prev):
                """DMA this block's NEW input rows, binarize into the padded
                bf16 tile; halo rows come from the previous block's tile.

                xb row k  <->  x row  blk*RB - 1 + k   (k = 0..33), so matmul
                for output row j, tap dy reads xb rows j+dy, j+dy+1.
                """
                r0 = blk * RB
                xin = xpool.tile([128, 33 * W], DT.float32, tag="xin")
                xin_v = xin[:].rearrange("p (r c) -> p r c", c=W)
                xb = xbpool.tile([128, ROWS_IN * WP], DT.bfloat16, tag="xb")
                xb_v = xb[:].rearrange("p (r c) -> p r c", c=WP)

                if blk == 0:
                    # xb rows 1..33  <-  x rows 0..32 (33 rows), split in two
                    # chunks so binarize+matmul can start before the second
                    # chunk lands.  xb row 0 = top pad.
                    for a, b in ((0, 17), (17, 33)):
                        nc.scalar.dma_start(
                            xin[:, a * W : b * W], x_flat[:, a * W : b * W]
                        )
                        nc.vector.tensor_scalar(
                            xb_v[:, 1 + a : 1 + b, 2 : 2 + W],
                            xin_v[:, a:b, :],
                            0.0,
                            None,
                            op0=mybir.AluOpType.is_ge,
                        )
                    nc.vector.memset(xb_v[:, 0:1, :], 0.5)
                    nc.vector.memset(xb_v[:, 1:34, 0:2], 0.5)
                    nc.vector.memset(xb_v[:, 1:34, 2 + W : WP], 0.5)
                else:
                    # halo: xb rows 0,1  <-  previous tile rows 32,33 (with
                    # their column pads already in place)
                    nc.vector.tensor_copy(xb_v[:, 0:2, :], xb_prev[:, 32:34, :])
                    # new rows r0+1 .. r0+32 (31 rows + pad for last block)
                    n_new = 31 if blk == NBLK - 1 else 32
                    nc.scalar.dma_start(
                        xin[:, 0 : n_new * W],
                        x_flat[:, (r0 + 1) * W : (r0 + 1 + n_new) * W],
                    )
                    nc.vector.tensor_scalar(
                        xb_v[:, 2 : 2 + n_new, 2 : 2 + W],
                        xin_v[:, 0:n_new, :],
                        0.0,
                        None,
                        op0=mybir.AluOpType.is_ge,
                    )
                    nc.vector.memset(xb_v[:, 2:34, 0:2], 0.5)
                    nc.vector.memset(xb_v[:, 2:34, 2 + W : WP], 0.5)
                    if blk == NBLK - 1:
                        nc.vector.memset(xb_v[:, 33:34, :], 0.5)
                return xb_v

            def compute_block(blk, xb_v):
                """Matmuls + post-ops + output DMAs for a loaded block."""
                r0 = blk * RB
                hb = RB // 2  # rows per half-block (16)
                # PSUM bank T = [imgA-top | imgB-top] (partition = n*64+c);
                # bank B = [imgB-bot | imgA-bot] (image-reversed; the reversal
                # is undone host-side).
                # Quadrants: A-T=(0,0)  B-T=(64,64)  B-B=(64,0)  A-B=(0,64)
                for it2 in range(hb // 4):          # 4 output rows per super-tile
                    # super-tiles spanning 2 PSUM banks; each matmul stays in one
                    ps_t = ppool.tile([128, 1024], DT.float32, tag="pst")
                    ps_b = ppool.tile([128, 1024], DT.float32, tag="psb")
                    for sub in range(2):            # 2 rows per matmul set
                        it = 2 * it2 + sub
                        c0 = sub * 512
                        for t in range(9):
                            dy, dx = divmod(t, 3)
                            first, last = (t == 0), (t == 8)
                            rt = 2 * it + dy              # top-half rows
                            rb_ = hb + 2 * it + dy        # bottom-half rows
                            cs = 1 + dx
                            quads = (
                                (ps_t, 0, 0, rt),      # A-top -> psT[0:64]
                                (ps_t, 64, 64, rt),    # B-top -> psT[64:128]
                                (ps_b, 64, 0, rb_),    # B-bot -> psB[0:64]
                                (ps_b, 0, 64, rb_),    # A-bot -> psB[64:128]
                            )
                            for ps, xp0, op0_, rlo in quads:
                                wslc = wt[xp0 : xp0 + 64, t * 64 : (t + 1) * 64]
                                rhs = xb_v[xp0 : xp0 + 64, rlo : rlo + 2, cs : cs + W]
                                nc.tensor.matmul(
                                    ps[op0_ : op0_ + 64, c0 : c0 + 512],
                                    wslc,
                                    rhs,
                                    start=first,
                                    stop=last,
                                )
                    # drain both banks: relu+bias (ACT), BN affine (DVE) into a
                    # per-supertile staging tile, then 1 MB output DMA (SP ring)
                    for ps, roff in ((ps_t, 4 * it2), (ps_b, hb + 4 * it2)):
                        tsb = tpool.tile([128, 1024], DT.float32, tag="tsb")
                        nc.scalar.activation(
                            tsb[:],
                            ps[:],
                            mybir.ActivationFunctionType.Relu,
                            bias=bv[:],
                            scale=2.0,
                        )
                        yst = ypool.tile([128, 1024], DT.float32, tag="yst")
                        nc.vector.tensor_scalar(
                            yst[:],
                            tsb[:],
                            iv[:],
                            cv[:],
                            op0=mybir.AluOpType.mult,
                            op1=mybir.AluOpType.add,
                        )
                        nc.sync.dma_start(
                            y_m[:, (r0 + roff) * W : (r0 + roff + 4) * W], yst[:]
                        )

            # software pipeline: queue block i+1's input DMA before block i's
            # output DMAs so input transfer overlaps compute
            pending = None
            prev_xb = None
            for blk in range(NBLK):
                xb_v = load_block(blk, prev_xb)
                if blk == 0:
                    load_consts()
                if pending is not None:
                    compute_block(pending[0], pending[1])
                pending = (blk, xb_v)
                prev_xb = xb_v
            compute_block(pending[0], pending[1])
    nc.compile()
    return nc


def _get_program():
    global _PROGRAM
    if _PROGRAM is None:
        _PROGRAM = _build()
    return _PROGRAM


def _prep_params(w, b, gamma, beta, running_mean, running_var):
    wb = np.where(w >= 0, 1.0, -1.0).astype(np.float32)          # [co, ci, ky, kx]
    wt = np.ascontiguousarray(wb.transpose(1, 2, 3, 0))          # [ci, ky, kx, co]
    wt = wt.reshape(C, 9 * C).astype(ml_dtypes.bfloat16)
    wt2 = np.ascontiguousarray(np.concatenate([wt, wt], axis=0))  # [128, 576]
    s = wb.sum(axis=(1, 2, 3)).astype(np.float32)
    inv = (gamma.astype(np.float32) / np.sqrt(running_var.astype(np.float32) + BN_EPS)).astype(np.float32)
    cc = (beta.astype(np.float32) - running_mean.astype(np.float32) * inv).astype(np.float32)
    bp = (b.astype(np.float32) - s).astype(np.float32)

    def rep(v):
        return np.ascontiguousarray(np.tile(v.astype(np.float32), 2).reshape(128, 1))

    return wt2, rep(bp), rep(inv), rep(cc)


def _unswizzle(yd):
    """Undo the on-device image swap of the bottom 16-row half of each
    32-row block (PSUM bank B holds image-reversed partitions)."""
    v = yd.reshape(IMGS_PER_CORE, C, NBLK, 2, RB // 2, W)
    out = np.empty_like(v)
    out[:, :, :, 0] = v[:, :, :, 0]
    out[:, :, :, 1] = v[::-1, :, :, 1]
    return out.reshape(IMGS_PER_CORE, C, H, W)


def run(x, w, b, gamma, beta, running_mean, running_var, trace=False):
    nc = _get_program()
    wt2, bp, inv, cc = _prep_params(w, b, gamma, beta, running_mean, running_var)
    x = np.asarray(x, dtype=np.float32)
    in_maps = []
    for i in range(N_CORES):
        in_maps.append(
            {
                "x": np.ascontiguousarray(x[IMGS_PER_CORE * i : IMGS_PER_CORE * (i + 1)]),
                "wT": wt2,
                "bvec": bp,
                "ivec": inv,
                "cvec": cc,
            }
        )
    res = run_bass_kernel_spmd(nc, in_maps, list(range(N_CORES)), trace=trace)
    y = np.concatenate(
        [_unswizzle(res.results[i]["y"]) for i in range(N_CORES)], axis=0
    )
    return y, res


def kernel(x, w, b, gamma, beta, running_mean, running_var):
    y, _ = run(x, w, b, gamma, beta, running_mean, running_var)
    return y
